# revision 1
# baseline (speedup 1.0000x reference)
"""Trainium2 Bass kernel for nn_CACE_LR (CACE message-passing GNN energy model).

Strategy (data parallel, 8 NeuronCores):
- Nodes split into 8 contiguous shards of 1250 (10 blocks of 128); edges
  with fc(r)=0 dropped on host; rest grouped by dst block, padded to a
  common per-block chunk count (SPMD).
- The radial transform is linear and commutes with the scatter-sum, so it
  is folded into per-edge features (6->24 map via fused scalar_tensor_tensor).
- Destination-embedding factorization: all edges in a chunk share their
  dst block, so edge_code = emb_i(src)*emb_j(dst) splits: the scatter runs
  over 360-wide (i,a,s) features; emb_j(dst) is applied per node at drain.
  Memory rows accumulate the same way into an SBUF-resident 360-wide hat.
- Stage-1 scatter matmuls in exact fp32 (PE has slack); stage-2 scatter in
  fp16 (one rounding; feeds only the message terms).
- chi2 = emb_i(n)*chi(n) is stored in the fp16 gather table so stage 2
  needs no angcode/code at all.
- Stage-2 A[src] rows fetched per chunk by one indirect DMA from a fp16
  table: 3 piecewise AllGathers (overlapped with stage-1 tail) are
  concatenated by DRAM-DRAM DMAs into one gatherable tensor.
- Per-core partial energies [16] summed on host.
"""
import sys
import types
import numpy as np
from math import factorial

# ---------------- static model config (mirrors reference) ----------------
MAX_L = 3
CUTOFF, PPOW = 5.5, 6
N_NODES, N_EDGES, N_GRAPHS = 10000, 80000, 16
MP_NORM = 1.0 / np.sqrt(10.0)

LXLYLZ = [(lx, ly, l - lx - ly) for l in range(MAX_L + 1)
          for lx in range(l, -1, -1) for ly in range(l - lx, -1, -1)]
MONO = np.array(LXLYLZ, np.int32)
L_OF = MONO.sum(1)
MIDX = {tuple(m): i for i, m in enumerate(LXLYLZ)}

def _mult(m):
    return factorial(sum(m)) / (factorial(m[0]) * factorial(m[1]) * factorial(m[2]))
MULT = np.array([_mult(m) for m in LXLYLZ], np.float32)

NU3_12 = {}
for m1 in [m for m in LXLYLZ if sum(m) == 1]:
    for m2 in [m for m in LXLYLZ if sum(m) == 2]:
        m12 = (m1[0] + m2[0], m1[1] + m2[1], m1[2] + m2[2])
        NU3_12[(MIDX[tuple(m1)], MIDX[tuple(m2)])] = MIDX[m12]

NCORES = 8
NPC = N_NODES // NCORES          # 1250
NBLK = (NPC + 127) // 128        # 10
ROWPC = NBLK * 128               # 1280
C, A, S, R = 9, 20, 6, 6
FW = C * A * S                   # 1080; feature row index (c, a, s)
HW = 360                         # hat-feature width (i, a, s)
CATW = 1092                      # FW + 9 chi2 + 3 pad
SUP = 8                          # chunks per super-group
EBW = 14                         # eb floats per edge: ps pd sh mk dl es
SQ2C = float(np.sqrt(2.0 / CUTOFF))
NZ = 94
# l blocks inside the a axis (a is l-major: [0], [1..3], [4..9], [10..19])
LBLK = [(0, 0, 1), (1, 1, 4), (2, 4, 10), (3, 10, 20)]
# AllGather pieces (local row ranges, one collective each as blocks finish)
PIECES = [(0, 512), (512, 768), (768, 1024), (1024, 1280)]

DEBUG = False


# ---------------- harness shims ----------------

def _install_ntff_shim():
    try:
        import antenv  # noqa
        if "antenv.axon_hooks" in sys.modules:
            return
        hooks_mod = types.ModuleType("antenv.axon_hooks")
        _hook = [None]
        hooks_mod.set_axon_ntff_profile_hook = lambda h: _hook.__setitem__(0, h)
        hooks_mod.get_axon_ntff_profile_hook = lambda: _hook[0]
        sys.modules["antenv.axon_hooks"] = hooks_mod
        antenv.axon_hooks = hooks_mod
        try:
            from trn_agent_boot.trn_boot import _ntff_profile_via_ctypes
            hooks_mod.set_axon_ntff_profile_hook(
                _ntff_profile_via_ctypes('/opt/axon/libaxon_pjrt.so'))
        except Exception:
            pass
    except Exception:
        pass


def _split_waits(nc, mybir, maxw=1):
    """This toolchain's walrus encodes at most one sync-wait per instruction;
    move extra waits onto preceding NOPs on the same engine."""
    cnt = 0
    for blk in nc.m.functions[0].blocks:
        out, changed = [], False
        for ins in blk.instructions:
            si = ins.sync_info
            if si is not None and len(si.on_wait) > maxw:
                waits = list(si.on_wait)
                extra, keep = waits[:-maxw], waits[-maxw:]
                while extra:
                    take, extra = extra[:maxw], extra[maxw:]
                    nop = mybir.InstNoOp(name=f"WSPLIT-{cnt}", ins=[], outs=[])
                    cnt += 1
                    nop.engine = ins.engine
                    nop.sync_info = mybir.SyncInfo(on_wait=take, on_update=[])
                    out.append(nop)
                ins.sync_info = mybir.SyncInfo(on_wait=keep,
                                               on_update=list(si.on_update))
                changed = True
            out.append(ins)
        if changed:
            blk.instructions = out
    return cnt


# ---------------- host-side sharding / staging ----------------


def host_prepare(pos, node_type, src, dst, shifts, batch_ids, Wemb):
    pos = np.ascontiguousarray(pos, np.float32)
    shifts = np.ascontiguousarray(shifts, np.float32)
    src = np.ascontiguousarray(src).astype(np.int64)
    dst = np.ascontiguousarray(dst).astype(np.int64)
    node_type = np.ascontiguousarray(node_type).astype(np.int64)
    batch_ids = np.ascontiguousarray(batch_ids).astype(np.int64)
    Wemb = np.ascontiguousarray(Wemb, np.float32)
    emb_nodes = Wemb[node_type]                       # [N, 3]

    vec = pos[dst] - pos[src] + shifts
    r = np.sqrt((vec * vec).sum(1))
    keep = r < CUTOFF                     # fc == 0 exactly for r >= CUTOFF
    ek = np.nonzero(keep)[0]
    owner = dst[ek] // NPC

    per_core_runs = []
    KB = np.zeros(NBLK, np.int64)
    for i in range(NCORES):
        sel = ek[owner == i]
        sel = sel[np.argsort(dst[sel], kind="stable")]
        blk = (dst[sel] - i * NPC) // 128
        runs = [sel[blk == b] for b in range(NBLK)]
        per_core_runs.append(runs)
        for b in range(NBLK):
            KB[b] = max(KB[b], (len(runs[b]) + 127) // 128)
    KB = np.maximum(KB, 1)
    KCH = int(KB.sum())
    EPAD = KCH * 128
    chunk_blk = np.concatenate(
        [[b] * int(KB[b]) for b in range(NBLK)]).astype(np.int64)

    supers = []
    c0 = 0
    while c0 < KCH:
        w = min(SUP, KCH - c0)
        supers.append((c0, w))
        c0 += w

    shards = []
    for i in range(NCORES):
        epos_s = np.zeros((EPAD, 3), np.float32)
        epos_d = np.zeros((EPAD, 3), np.float32)
        eshift = np.zeros((EPAD, 3), np.float32)
        emask = np.zeros((EPAD,), np.float32)
        edloc = np.zeros((EPAD,), np.float32)
        esrc = np.zeros((EPAD, 3), np.float32)
        esrcrow = np.zeros((EPAD,), np.int32)
        off = 0
        for b in range(NBLK):
            run = per_core_runs[i][b]
            m = len(run)
            sl = slice(off, off + m)
            epos_s[sl] = pos[src[run]]
            epos_d[sl] = pos[dst[run]]
            eshift[sl] = shifts[run]
            emask[sl] = 1.0
            edloc[sl] = (dst[run] - i * NPC - b * 128).astype(np.float32)
            esrc[sl] = emb_nodes[src[run]]
            esrcrow[sl] = ((src[run] // NPC) * ROWPC
                           + (src[run] % NPC)).astype(np.int32)
            off += int(KB[b]) * 128

        def wrap(x, w=1):
            return np.ascontiguousarray(
                x.reshape(KCH, 128, w).transpose(1, 0, 2))

        ps = wrap(epos_s, 3); pd = wrap(epos_d, 3); sh = wrap(eshift, 3)
        mk = wrap(emask[:, None], 1); dl = wrap(edloc[:, None], 1)
        es = wrap(esrc, 3)
        eb = np.zeros((128, EBW * KCH), np.float32)
        for (c0_, w) in supers:
            base = EBW * c0_
            eb[:, base: base + 3 * w] = ps[:, c0_:c0_ + w].reshape(128, 3 * w)
            eb[:, base + 3 * w: base + 6 * w] = pd[:, c0_:c0_ + w].reshape(128, 3 * w)
            eb[:, base + 6 * w: base + 9 * w] = sh[:, c0_:c0_ + w].reshape(128, 3 * w)
            eb[:, base + 9 * w: base + 10 * w] = mk[:, c0_:c0_ + w, 0]
            eb[:, base + 10 * w: base + 11 * w] = dl[:, c0_:c0_ + w, 0]
            eb[:, base + 11 * w: base + 14 * w] = es[:, c0_:c0_ + w].reshape(128, 3 * w)

        bid = np.full((128, NBLK), -1.0, np.float32)
        bl = batch_ids[i * NPC:(i + 1) * NPC].astype(np.float32)
        embn = np.zeros((128, NBLK * 3), np.float32)
        for b in range(NBLK):
            n = min(128, NPC - b * 128)
            bid[:n, b] = bl[b * 128: b * 128 + n]
            embn[:n, 3 * b:3 * b + 3] = emb_nodes[i * NPC + b * 128:
                                                  i * NPC + b * 128 + n]

        shards.append(dict(
            eb=eb,
            srcrow=wrap(esrcrow[:, None], 1)[:, :, 0],
            bid=bid,
            embn=embn,
        ))
    return shards, chunk_blk, supers, KCH


def host_weights(Wemb, freqs, W_rt, W_mem, W_Ar, W_chi, W1, b1, W2, b2, W3, b3):
    freqs = np.ascontiguousarray(freqs, np.float32)
    W_rt = np.ascontiguousarray(W_rt, np.float32)
    W_mem = np.ascontiguousarray(W_mem, np.float32)
    W_Ar = np.ascontiguousarray(W_Ar, np.float32)
    W_chi = np.ascontiguousarray(W_chi, np.float32)
    W1 = np.ascontiguousarray(W1, np.float32)

    # wl_s1[r, (g,l,s)]: g=0 rt=W_rt, g=1 memfold=(W_rt@W_mem)
    w_mem_f = np.stack([W_rt[l] @ W_mem[l] for l in range(MAX_L + 1)])
    wl_s1 = np.stack([np.transpose(W_rt, (1, 0, 2)),
                      np.transpose(w_mem_f, (1, 0, 2))], axis=1)   # [R,2,4,S]
    # wl_s2[r, (g,l,s)]: g=0 mp=W_rt*MP, g=1 ar=W_Ar*MP
    wl_s2 = np.stack([np.transpose(W_rt * MP_NORM, (1, 0, 2)),
                      np.transpose(W_Ar * MP_NORM, (1, 0, 2))], axis=1) * 16.0
    wl1_rep = np.tile(wl_s1.reshape(1, R * 48), (128, 1)).astype(np.float32)
    wl2_rep = np.tile(wl_s2.reshape(1, R * 48), (128, 1)).astype(np.float32)

    permB = np.zeros(324, np.int64)
    for sym in range(6):
        for c in range(C):
            for s in range(S):
                permB[sym * 54 + c * 6 + s] = s * 54 + sym * 9 + c
    permF = np.zeros(648, np.int64)
    for t in range(2):
        permF[t * 324:(t + 1) * 324] = permB * 2 + t

    mrow = np.zeros(FW, np.float32)
    for c in range(C):
        for a in range(A):
            base = c * (A * S) + a * S
            mrow[base:base + S] = MULT[a]

    return dict(
        wl1=wl1_rep,
        wl2=wl2_rep,
        multrow=np.tile(mrow.reshape(1, FW), (128, 1)),
        freqs6=np.tile(freqs.reshape(1, 6), (128, 1)),
        wchi=np.ascontiguousarray(W_chi[permB]).astype(np.float32),
        w1=np.ascontiguousarray(W1[permF]).astype(np.float32),
        w2=np.ascontiguousarray(W2, np.float32),
        w3=np.ascontiguousarray(W3, np.float32),
        b1c=np.ascontiguousarray(b1, np.float32).reshape(64, 1),
        b2c=np.ascontiguousarray(b2, np.float32).reshape(32, 1),
        b3=float(np.asarray(b3).reshape(-1)[0]),
    )


# ---------------- device program ----------------


def build_program(chunk_blk, supers, KCH, b3val):
    import os as _os
    import concourse.bass as bass
    import concourse.mybir as mybir
    import concourse.tile as tile
    from concourse.masks import make_identity

    f32 = mybir.dt.float32
    bf16 = mybir.dt.bfloat16
    f16 = mybir.dt.float16
    i32 = mybir.dt.int32
    AF = mybir.ActivationFunctionType
    OP = mybir.AluOpType

    nc = bass.Bass(num_devices=NCORES)

    eb_d = nc.dram_tensor("eb", [128, EBW * KCH], f32, kind="ExternalInput")
    srcrow_d = nc.dram_tensor("srcrow", [128, KCH], i32, kind="ExternalInput")
    bid_d = nc.dram_tensor("bid", [128, NBLK], f32, kind="ExternalInput")
    embn_d = nc.dram_tensor("embn", [128, NBLK * 3], f32, kind="ExternalInput")
    wl1_d = nc.dram_tensor("wl1", [128, 288], f32, kind="ExternalInput")
    wl2_d = nc.dram_tensor("wl2", [128, 288], f32, kind="ExternalInput")
    multrow_d = nc.dram_tensor("multrow", [128, FW], f32, kind="ExternalInput")
    freqs6_d = nc.dram_tensor("freqs6", [128, 6], f32, kind="ExternalInput")
    wchi_d = nc.dram_tensor("wchi", [324, 9], f32, kind="ExternalInput")
    w1_d = nc.dram_tensor("w1", [648, 64], f32, kind="ExternalInput")
    w2_d = nc.dram_tensor("w2", [64, 32], f32, kind="ExternalInput")
    w3_d = nc.dram_tensor("w3", [32, 1], f32, kind="ExternalInput")
    b1c_d = nc.dram_tensor("b1c", [64, 1], f32, kind="ExternalInput")
    b2c_d = nc.dram_tensor("b2c", [32, 1], f32, kind="ExternalInput")
    energy_d = nc.dram_tensor("energy", [16, 1], f32, kind="ExternalOutput")
    if DEBUG:
        dbg_arow_d = nc.dram_tensor("dbg_arow", [128, NBLK * CATW], f32,
                                    kind="ExternalOutput")
        dbg_anew_d = nc.dram_tensor("dbg_anew", [128, NBLK * FW], f32,
                                    kind="ExternalOutput")

    last_chunk_of_block = {}
    first_chunk_of_block = {}
    for k, b in enumerate(chunk_blk):
        b = int(b)
        last_chunk_of_block[b] = k
        if b not in first_chunk_of_block:
            first_chunk_of_block[b] = k
    QUADS = [list(range(q, min(q + 4, NBLK))) for q in range(0, NBLK, 4)]
    quad_of_block = {}
    for qi, q in enumerate(QUADS):
        for b in q:
            quad_of_block[b] = qi

    _bisect = _os.environ.get("TRN_BISECT", "full")

    with tile.TileContext(nc) as tc:
        with tc.tile_pool(name="const", bufs=1) as constp, \
             tc.tile_pool(name="persist", bufs=1) as persist, \
             tc.tile_pool(name="edge", bufs=2) as edgep, \
             tc.tile_pool(name="gath", bufs=3) as gathp, \
             tc.tile_pool(name="blk", bufs=2) as blkp, \
             tc.tile_pool(name="quad", bufs=1) as quadp, \
             tc.tile_pool(name="psA", bufs=2, space="PSUM") as psA, \
             tc.tile_pool(name="psT", bufs=1, space="PSUM") as psT, \
             tc.tile_pool(name="dram", bufs=1, space="DRAM") as dramp:

            # ---- constants ----
            ident = constp.tile([128, 128], f32)
            make_identity(nc, ident[:])
            iota_s = constp.tile([128, SUP * 128], f32)
            nc.gpsimd.iota(iota_s[:], pattern=[[0, SUP], [1, 128]],
                           base=0, channel_multiplier=0,
                           allow_small_or_imprecise_dtypes=True)
            iota16 = constp.tile([128, 16], f32)
            nc.gpsimd.iota(iota16[:], pattern=[[1, 16]], base=0,
                           channel_multiplier=0,
                           allow_small_or_imprecise_dtypes=True)

            def const_load(name, dram, shape, dt=f32):
                t = constp.tile(shape, dt, name=name, tag=name)
                nc.sync.dma_start(t[:], dram[:])
                return t
            wl1_w = const_load("wl1", wl1_d, [128, 288])
            wl2_w = const_load("wl2", wl2_d, [128, 288])
            multrow_w = const_load("multrow", multrow_d, [128, FW])
            freqs6_w = const_load("freqs6", freqs6_d, [128, 6])
            w2_w = const_load("w2", w2_d, [64, 32])
            w3_w = const_load("w3", w3_d, [32, 1])
            b1c_w = const_load("b1c", b1c_d, [64, 1])
            b2c_w = const_load("b2c", b2c_d, [32, 1])
            bid_w = const_load("bid", bid_d, [128, NBLK])
            embn_w = const_load("embn", embn_d, [128, NBLK * 3])
            embn2_w = constp.tile([128, NBLK * 3], f32, name="embn2",
                                  tag="embn2")
            nc.vector.tensor_scalar_mul(embn2_w[:], embn_w[:],
                                        1.0 / 256.0)
            wchi_w = []
            for c3 in range(3):
                t = constp.tile([108, 9], f32, name=f"wchi{c3}",
                                tag=f"wchi{c3}")
                nc.sync.dma_start(t[:], wchi_d[108 * c3:108 * (c3 + 1), :])
                wchi_w.append(t)
            w1_w = []
            for c6 in range(6):
                t = constp.tile([108, 64], f32, name=f"w1_{c6}",
                                tag=f"w1_{c6}")
                nc.sync.dma_start(t[:], w1_d[108 * c6:108 * (c6 + 1), :])
                w1_w.append(t)
            srcrow_w = constp.tile([128, KCH], i32)
            nc.sync.dma_start(srcrow_w[:], srcrow_d[:])
            identb = constp.tile([128, 128], bf16)
            nc.vector.tensor_copy(identb[:], ident[:])

            h1_all = persist.tile([64, NBLK * 128], f32)
            radial_all = persist.tile([128, 6 * KCH], f32)
            unit_all = persist.tile([128, 3 * KCH], f32)
            memhat_all = persist.tile([128, NBLK * HW], f32)
            energy_sb = persist.tile([16, 1], f32)
            nc.vector.memset(energy_sb[:], 0.0)
            arow4 = persist.tile([128, 4 * CATW], f32)

            acatl = [dramp.tile([p1 - p0, CATW], f16, name=f"acatl{j}")
                     for j, (p0, p1) in enumerate(PIECES)]
            acatp = [dramp.tile([NCORES * (p1 - p0), CATW], f16,
                                name=f"acatp{j}", addr_space="Shared")
                     for j, (p0, p1) in enumerate(PIECES)]
            acatc = dramp.tile([NCORES * ROWPC, CATW], f16)

            def u1(ap):
                return ap.rearrange("p (a b) -> p a b", b=1)

            # -------- per-edge features for one super --------
            def edge_features(c0, w, stage2):
                W3c = 3 * w
                ebt = edgep.tile([128, EBW * SUP], f32, tag="ebt")
                if stage2:
                    nc.sync.dma_start(
                        ebt[:, :w],
                        eb_d[:, EBW * c0 + 10 * w: EBW * c0 + 11 * w])
                    dl = ebt[:, 0:w]
                    es = None
                else:
                    nc.sync.dma_start(ebt[:, :EBW * w],
                                      eb_d[:, EBW * c0: EBW * (c0 + w)])
                    ps = ebt[:, 0:W3c]
                    pd = ebt[:, W3c:2 * W3c]
                    sh = ebt[:, 2 * W3c:3 * W3c]
                    mk = ebt[:, 9 * w:10 * w]
                    dl = ebt[:, 10 * w:11 * w]
                    es = ebt[:, 11 * w:14 * w]

                Pm = edgep.tile([128, 128 * SUP], f32, tag="Pm")
                nc.vector.tensor_tensor(
                    Pm[:].rearrange("p (a b) -> p a b", b=128)[:, :w, :],
                    iota_s[:].rearrange("p (a b) -> p a b", b=128)[:, :w, :],
                    u1(dl).to_broadcast([128, w, 128]),
                    op=OP.is_equal)

                if stage2:
                    unitv = unit_all[:, 3 * c0: 3 * (c0 + w)] \
                        .rearrange("p (a b) -> p a b", b=3)
                else:
                    vec = edgep.tile([128, 3 * SUP], f32, tag="vec")
                    nc.vector.tensor_tensor(vec[:, :W3c], pd, sh, op=OP.add)
                    nc.vector.tensor_tensor(vec[:, :W3c], vec[:, :W3c], ps,
                                            op=OP.subtract)
                    sq = edgep.tile([128, 3 * SUP], f32, tag="sq")
                    nc.scalar.activation(sq[:, :W3c], vec[:, :W3c], AF.Square)
                    rr = edgep.tile([128, SUP], f32, tag="rr")
                    nc.vector.tensor_reduce(
                        rr[:, :w], sq[:, :W3c].rearrange("p (a b) -> p a b", b=3),
                        axis=mybir.AxisListType.X, op=OP.add)
                    r_ = edgep.tile([128, SUP], f32, tag="r_")
                    nc.scalar.activation(r_[:, :w], rr[:, :w], AF.Sqrt)
                    nc.vector.tensor_scalar_add(r_[:, :w], r_[:, :w], 1e-9)
                    invr = edgep.tile([128, SUP], f32, tag="invr")
                    nc.vector.reciprocal(invr[:, :w], r_[:, :w])
                    unit = unit_all[:, 3 * c0: 3 * (c0 + w)]
                    unitv = unit.rearrange("p (a b) -> p a b", b=3)
                    nc.vector.tensor_tensor(
                        unitv,
                        vec[:, :W3c].rearrange("p (a b) -> p a b", b=3),
                        u1(invr[:, :w]).to_broadcast([128, w, 3]),
                        op=OP.mult)

                    arg = edgep.tile([128, 6 * SUP], f32, tag="arg")
                    nc.vector.tensor_tensor(
                        arg[:, :6 * w].rearrange("p (a b) -> p a b", b=6),
                        freqs6_w[:].rearrange("p (x q) -> p x q", x=1)
                        .to_broadcast([128, w, 6]),
                        u1(r_[:, :w]).to_broadcast([128, w, 6]),
                        op=OP.mult)
                    karg = edgep.tile([128, 6 * SUP], mybir.dt.int32, tag="karg")
                    nc.vector.tensor_scalar(arg[:, :6 * w], arg[:, :6 * w],
                                            float(1.0 / (2 * np.pi)), None,
                                            op0=OP.mult)
                    nc.vector.tensor_copy(karg[:, :6 * w], arg[:, :6 * w])
                    kf = edgep.tile([128, 6 * SUP], f32, tag="kf")
                    nc.vector.tensor_copy(kf[:, :6 * w], karg[:, :6 * w])
                    nc.vector.tensor_tensor(arg[:, :6 * w], arg[:, :6 * w],
                                            kf[:, :6 * w], op=OP.subtract)
                    nc.vector.tensor_scalar(arg[:, :6 * w], arg[:, :6 * w],
                                            float(2 * np.pi), None, op0=OP.mult)
                    sin_t = edgep.tile([128, 6 * SUP], f32, tag="sin_t")
                    nc.scalar.activation(sin_t[:, :6 * w], arg[:, :6 * w], AF.Sin)
                    rbf0 = edgep.tile([128, 6 * SUP], f32, tag="rbf0")
                    nc.vector.tensor_tensor(
                        rbf0[:, :6 * w].rearrange("p (a b) -> p a b", b=6),
                        sin_t[:, :6 * w].rearrange("p (a b) -> p a b", b=6),
                        u1(invr[:, :w]).to_broadcast([128, w, 6]),
                        op=OP.mult)

                    u = edgep.tile([128, SUP], f32, tag="u")
                    nc.vector.tensor_scalar_mul(u[:, :w], r_[:, :w], 1.0 / CUTOFF)
                    u3 = edgep.tile([128, SUP], f32, tag="u3")
                    nc.vector.tensor_tensor(u3[:, :w], u[:, :w], u[:, :w],
                                            op=OP.mult)
                    nc.vector.tensor_tensor(u3[:, :w], u3[:, :w], u[:, :w],
                                            op=OP.mult)
                    u6 = edgep.tile([128, SUP], f32, tag="u6")
                    nc.vector.tensor_tensor(u6[:, :w], u3[:, :w], u3[:, :w],
                                            op=OP.mult)
                    fc = edgep.tile([128, SUP], f32, tag="fc")
                    c6_ = -(PPOW + 1) * (PPOW + 2) / 2.0
                    c7_ = float(PPOW * (PPOW + 2))
                    c8_ = -PPOW * (PPOW + 1) / 2.0
                    nc.vector.tensor_scalar(fc[:, :w], u6[:, :w], c6_, 1.0,
                                            op0=OP.mult, op1=OP.add)
                    t7 = edgep.tile([128, SUP], f32, tag="t7")
                    nc.vector.tensor_tensor(t7[:, :w], u6[:, :w], u[:, :w],
                                            op=OP.mult)
                    u8 = edgep.tile([128, SUP], f32, tag="u8")
                    nc.vector.tensor_tensor(u8[:, :w], t7[:, :w], u[:, :w],
                                            op=OP.mult)
                    nc.vector.tensor_scalar_mul(t7[:, :w], t7[:, :w], c7_)
                    nc.vector.tensor_tensor(fc[:, :w], fc[:, :w], t7[:, :w],
                                            op=OP.add)
                    nc.vector.tensor_scalar_mul(u8[:, :w], u8[:, :w], c8_)
                    nc.vector.tensor_tensor(fc[:, :w], fc[:, :w], u8[:, :w],
                                            op=OP.add)
                    gate = edgep.tile([128, SUP], f32, tag="gate")
                    nc.vector.tensor_scalar(gate[:, :w], u[:, :w], 1.0, None,
                                            op0=OP.is_lt)
                    nc.vector.tensor_tensor(fc[:, :w], fc[:, :w], gate[:, :w],
                                            op=OP.mult)
                    nc.vector.tensor_tensor(fc[:, :w], fc[:, :w], mk, op=OP.mult)
                    nc.vector.tensor_scalar_mul(fc[:, :w], fc[:, :w], SQ2C)

                    radial = radial_all[:, 6 * c0: 6 * (c0 + w)]
                    nc.vector.tensor_tensor(
                        radial.rearrange("p (a b) -> p a b", b=6),
                        rbf0[:, :6 * w].rearrange("p (a b) -> p a b", b=6),
                        u1(fc[:, :w]).to_broadcast([128, w, 6]),
                        op=OP.mult)

                # angular monomials (both stages; cheap)
                ang = edgep.tile([128, 20 * SUP], f32, tag="ang")
                angv = ang[:].rearrange("p (a b) -> p a b", b=20)
                nc.vector.memset(angv[:, :w, 0:1], 1.0)
                nc.vector.tensor_copy(angv[:, :w, 1:4], unitv[:, :, :])
                xcol = unitv[:, :, 0:1]
                ycol = unitv[:, :, 1:2]
                zcol = unitv[:, :, 2:3]
                nc.vector.tensor_tensor(angv[:, :w, 4:7], unitv[:, :, :],
                                        xcol.to_broadcast([128, w, 3]), op=OP.mult)
                nc.vector.tensor_tensor(angv[:, :w, 7:9], unitv[:, :, 1:3],
                                        ycol.to_broadcast([128, w, 2]), op=OP.mult)
                nc.vector.tensor_tensor(angv[:, :w, 9:10], zcol, zcol, op=OP.mult)
                nc.vector.tensor_tensor(angv[:, :w, 10:16], angv[:, :w, 4:10],
                                        xcol.to_broadcast([128, w, 6]), op=OP.mult)
                nc.vector.tensor_tensor(angv[:, :w, 16:19], angv[:, :w, 7:10],
                                        ycol.to_broadcast([128, w, 3]), op=OP.mult)
                nc.vector.tensor_tensor(angv[:, :w, 19:20], angv[:, :w, 9:10],
                                        zcol, op=OP.mult)

                # radial maps: radl[(g,l,s)=48] per chunk via fused STT, then
                # l->a expansion on ACT, [128, (g,a,s)=240] per chunk
                wsrc = wl1_w if not stage2 else wl2_w
                radl = edgep.tile([128, 48 * SUP], f32, tag="radl")
                rtmp = edgep.tile([128, 48 * 6], f32, tag="rtmp")
                for cl in range(w):
                    k = c0 + cl
                    dstr = radl[:, 48 * cl:48 * (cl + 1)]
                    for r in range(6):
                        rcol = radial_all[:, 6 * k + r: 6 * k + r + 1]
                        tgt = dstr if r == 0 else rtmp[:, 48 * r:48 * (r + 1)]
                        nc.scalar.activation(tgt, wsrc[:, 48 * r:48 * (r + 1)],
                                             AF.Copy, scale=rcol)
                    for r in range(1, 6):
                        eng = nc.vector if r % 2 == 0 else nc.gpsimd
                        eng.tensor_tensor(dstr, dstr,
                                          rtmp[:, 48 * r:48 * (r + 1)],
                                          op=OP.add)
                radx = edgep.tile([128, 240 * SUP], f32, tag="radx")
                rx6 = radx[:].rearrange("p (a g k s) -> p a g k s",
                                        g=2, k=20, s=6)
                rl6 = radl[:].rearrange("p (a g l s) -> p a g l s",
                                        g=2, l=4, s=6)
                for g2 in range(2):
                    for (l, a0, a1) in LBLK:
                        nc.scalar.copy(
                            rx6[:, :w, g2, a0:a1, :],
                            rl6[:, :w, g2, l:l + 1, :]
                            .to_broadcast([128, w, a1 - a0, 6]))

                return dict(Pm=Pm, ang=ang, angv=angv, radx=radx[:],
                            rx6=rx6, es=es, dl=dl)

            # -------- quad-batched symmetrize: arows -> brows --------
            def symmetrize_quad(arows, nb, brows, stride=CATW):
                def view(t, off, st):
                    return bass.AP(t.tensor, t.offset + off,
                                   [t.ap[0], [st, nb], [120, 9], [1, 6]])
                SQM = quadp.tile([128, 4 * FW], f32, tag="SQM")
                Asc = quadp.tile([128, 4 * FW], f32, tag="Asc")
                for x in range(nb):
                    aro = arows[:, stride * x: stride * x + FW]
                    nc.gpsimd.tensor_tensor(Asc[:, FW * x:FW * (x + 1)],
                                            aro, multrow_w[:], op=OP.mult)
                    nc.vector.tensor_tensor(SQM[:, FW * x:FW * (x + 1)],
                                            aro, Asc[:, FW * x:FW * (x + 1)],
                                            op=OP.mult)
                AV = lambda a: view(arows, a * S, stride)
                QV = lambda a: view(SQM[:], a * S, FW)
                CV = lambda a: view(Asc[:], a * S, FW)
                bview = brows.rearrange("p (x y c s) -> p x y c s", x=nb, y=6,
                                        s=S)
                BV = lambda y: bview[:, :, y, :, :]
                nc.scalar.copy(BV(0), AV(0))
                for li, (a0, a1) in enumerate([(1, 4), (4, 10), (10, 20)]):
                    dst = BV(1 + li)
                    eng0 = nc.vector if li % 2 == 0 else nc.gpsimd
                    eng0.tensor_tensor(dst, QV(a0), QV(a0 + 1), op=OP.add)
                    for a_ in range(a0 + 2, a1):
                        eng = nc.vector if a_ % 2 == 0 else nc.gpsimd
                        eng.tensor_tensor(dst, dst, QV(a_), op=OP.add)
                t54 = quadp.tile([128, 4 * 54], f32, tag="t54")
                u54 = quadp.tile([128, 4 * 54], f32, tag="u54")
                t54v = t54[:, :54 * nb].rearrange("p (x c s) -> p x c s",
                                                  x=nb, s=S)
                u54v = u54[:, :54 * nb].rearrange("p (x c s) -> p x c s",
                                                  x=nb, s=S)
                dstB = BV(4)
                first = True
                for (i_, ii) in [(1, 4), (2, 7), (3, 9)]:
                    tgt = dstB if first else t54v
                    nc.vector.tensor_tensor(tgt, QV(i_), CV(ii), op=OP.mult)
                    if not first:
                        nc.vector.tensor_tensor(dstB, dstB, t54v, op=OP.add)
                    first = False
                for (i_, j_, ij) in [(1, 2, 5), (1, 3, 6), (2, 3, 8)]:
                    nc.gpsimd.tensor_tensor(t54v, AV(i_), AV(j_), op=OP.mult)
                    nc.gpsimd.tensor_tensor(t54v, t54v, CV(ij), op=OP.mult)
                    nc.vector.tensor_scalar_mul(t54[:, :54 * nb], t54[:, :54 * nb],
                                                2.0)
                    nc.vector.tensor_tensor(dstB, dstB, t54v, op=OP.add)
                dstB2 = BV(5)
                firstm = True
                for m1 in (1, 2, 3):
                    firsti = True
                    for m2 in range(4, 10):
                        i12 = NU3_12[(m1, m2)]
                        eng = nc.gpsimd if (m2 % 2 == 0) else nc.vector
                        eng.tensor_tensor(u54v if firsti else t54v,
                                          AV(m2), CV(i12), op=OP.mult)
                        if not firsti:
                            nc.vector.tensor_tensor(u54v, u54v, t54v, op=OP.add)
                        firsti = False
                    nc.vector.tensor_tensor(u54v, u54v, AV(m1), op=OP.mult)
                    if firstm:
                        nc.vector.tensor_copy(dstB2, u54v)
                    else:
                        nc.vector.tensor_tensor(dstB2, dstB2, u54v, op=OP.add)
                    firstm = False

            # -------- B^T, chi, h1 (per block) --------
            def bt_compute(brow, b, stage, acrow=None):
                bts = []
                for c3 in range(3):
                    btp = psT.tile([128, 128], f32, tag="psbt", name="btp")
                    nc.tensor.transpose(btp[:108, :],
                                        brow[:, 108 * c3:108 * (c3 + 1)],
                                        ident[:])
                    bts_ = blkp.tile([108, 128], f32, tag=f"bts{c3}",
                                     name=f"bts{c3}")
                    nc.scalar.copy(bts_[:], btp[:108, :])
                    bts.append(bts_)
                h1p = psT.tile([64, 128], f32, tag="ps1", name="h1p")
                for c3 in range(3):
                    nc.tensor.matmul(h1p[:], w1_w[3 * stage + c3][:], bts[c3][:],
                                     start=(c3 == 0), stop=(c3 == 2))
                if stage == 0:
                    nc.vector.tensor_copy(h1_all[:, 128 * b:128 * (b + 1)],
                                          h1p[:])
                    chip = psT.tile([16, 128], f32, tag="ps1", name="chip")
                    for c3 in range(3):
                        nc.tensor.matmul(chip[:9, :], wchi_w[c3][:], bts[c3][:],
                                         start=(c3 == 0), stop=(c3 == 2))
                    chis = blkp.tile([9, 128], f32, tag="chis")
                    nc.scalar.copy(chis[:], chip[:9, :])
                    chirp = psT.tile([128, 16], f32, tag="ps1", name="chirp")
                    nc.tensor.transpose(chirp[:, :9], chis[:], ident[:9, :9])
                    # chi2[(i,j)] = emb_i(n) * chi[(i,j)]
                    nc.vector.tensor_tensor(
                        acrow[:, FW:FW + 9].rearrange("p (i j) -> p i j", i=3),
                        chirp[:, :9].rearrange("p (i j) -> p i j", i=3),
                        embn_w[:, 3 * b:3 * b + 3]
                        .rearrange("p (i x) -> p i x", x=1)
                        .to_broadcast([128, 3, 3]),
                        op=OP.mult)
                    return None
                h1f = blkp.tile([64, 128], f32, tag="h1f")
                nc.vector.tensor_tensor(h1f[:], h1p[:],
                                        h1_all[:, 128 * b:128 * (b + 1)],
                                        op=OP.add)
                return h1f

            # ================= STAGE 1 =================
            psum1 = {}
            for (c0, w) in supers:
                ef = edge_features(c0, w, stage2=False)
                esv = ef["es"].rearrange("p (a b) -> p a b", b=3)
                for cl in range(w):
                    k = c0 + cl
                    b = int(chunk_blk[k])
                    # y1 = es (x) ang*radx_rt ; ym = es (x) ang*radx_mem
                    agl = edgep.tile([128, 240], f32, tag="agl", bufs=3)
                    aglv = agl[:].rearrange("p (g k s) -> p g k s", g=2, s=6)
                    nc.vector.tensor_tensor(
                        aglv[:, 0], ef["rx6"][:, cl, 0],
                        ef["angv"][:, cl].rearrange("p (k x) -> p k x", x=1)
                        .to_broadcast([128, 20, 6]),
                        op=OP.mult)
                    nc.gpsimd.tensor_tensor(
                        aglv[:, 1], ef["rx6"][:, cl, 1],
                        ef["angv"][:, cl].rearrange("p (k x) -> p k x", x=1)
                        .to_broadcast([128, 20, 6]),
                        op=OP.mult)
                    y1 = edgep.tile([128, HW], f32, tag="y1", bufs=3)
                    ym = edgep.tile([128, HW], f32, tag="ym", bufs=3)
                    escol = esv[:, cl]
                    nc.vector.tensor_tensor(
                        y1[:].rearrange("p (i q) -> p i q", i=3),
                        escol.rearrange("p (i x) -> p i x", x=1)
                        .to_broadcast([128, 3, 120]),
                        agl[:, 0:120].rearrange("p (x q) -> p x q", x=1)
                        .to_broadcast([128, 3, 120]),
                        op=OP.mult)
                    nc.gpsimd.tensor_tensor(
                        ym[:].rearrange("p (i q) -> p i q", i=3),
                        escol.rearrange("p (i x) -> p i x", x=1)
                        .to_broadcast([128, 3, 120]),
                        agl[:, 120:240].rearrange("p (x q) -> p x q", x=1)
                        .to_broadcast([128, 3, 120]),
                        op=OP.mult)
                    if k == first_chunk_of_block[b]:
                        psum1[b] = [psA.tile([128, HW], f32, tag=f"sc{i_}",
                                             name=f"ps1_{i_}")
                                    for i_ in range(2)]
                    st = (k == first_chunk_of_block[b])
                    sp = (k == last_chunk_of_block[b])
                    nc.tensor.matmul(psum1[b][0][:],
                                     ef["Pm"][:, 128 * cl:128 * (cl + 1)],
                                     y1[:], start=st, stop=sp)
                    nc.tensor.matmul(psum1[b][1][:],
                                     ef["Pm"][:, 128 * cl:128 * (cl + 1)],
                                     ym[:], start=st, stop=sp)
                    if not sp:
                        continue
                    # ---- per-block drain: expand with emb_j(dst) ----
                    b_ = b
                    x4 = b_ % 4
                    acrow = arow4[:, CATW * x4: CATW * (x4 + 1)]
                    nc.scalar.copy(memhat_all[:, HW * b_:HW * (b_ + 1)],
                                   psum1[b_][1][:])
                    for j in range(3):
                        ej = embn_w[:, 3 * b_ + j:3 * b_ + j + 1]
                        dstv = bass.AP(acrow.tensor,
                                       acrow.offset + j * 120,
                                       [acrow.ap[0], [360, 3], [1, 120]])
                        nc.vector.tensor_scalar(
                            dstv,
                            psum1[b_][0][:].rearrange("p (i q) -> p i q", i=3),
                            ej, None, op0=OP.mult)
                    # ---- quad node phase ----
                    if b_ == QUADS[quad_of_block[b_]][-1]:
                        q = QUADS[quad_of_block[b_]]
                        nb = len(q)
                        brows = quadp.tile([128, 4 * 324], f32, tag="brows")
                        symmetrize_quad(arow4[:, :nb * CATW], nb,
                                        brows[:, :nb * 324], stride=CATW)
                        for xi, bb in enumerate(q):
                            bt_compute(brows[:, 324 * xi:324 * (xi + 1)], bb,
                                       stage=0,
                                       acrow=arow4[:, CATW * xi:CATW * (xi + 1)])
                            castrow = blkp.tile([128, CATW], f16,
                                                tag="castrow")
                            nc.scalar.activation(
                                castrow[:, :FW + 9],
                                arow4[:, CATW * xi: CATW * xi + FW + 9],
                                AF.Copy, scale=16.0)
                            nc.vector.memset(castrow[:, FW + 9:], 0.0)
                            pj = next(jj for jj, (pp0, pp1)
                                      in enumerate(PIECES)
                                      if pp0 <= 128 * bb < pp1)
                            pb = 128 * bb - PIECES[pj][0]
                            nc.sync.dma_start(
                                acatl[pj][pb:pb + 128, :],
                                castrow[:])
                            if (_bisect in ("s1c", "full")
                                    and 128 * (bb + 1) == PIECES[pj][1]):
                                nc.gpsimd.collective_compute(
                                    "AllGather", mybir.AluOpType.bypass,
                                    replica_groups=[list(range(NCORES))],
                                    ins=[acatl[pj][:].opt()],
                                    outs=[acatp[pj][:].opt()],
                                )
                            if DEBUG:
                                nc.sync.dma_start(
                                    dbg_arow_d[:, CATW * bb:CATW * (bb + 1)],
                                    arow4[:, CATW * xi: CATW * (xi + 1)])

            # concat the gathered pieces into one table (DMA queues, overlaps)
            if _bisect in ("s1c", "full"):
                accv = acatc[:].rearrange("(n r) w -> n r w", n=NCORES)
                for j, (p0, p1) in enumerate(PIECES):
                    psz = p1 - p0
                    nc.sync.dma_start(
                        accv[:, p0:p1, :],
                        acatp[j][:].rearrange("(n r) w -> n r w", n=NCORES))

            # ================= STAGE 2 =================
            psum2 = {}
            for (c0, w) in (supers if _bisect == "full" else []):
                ef = edge_features(c0, w, stage2=True)
                for cl in range(w):
                    k = c0 + cl
                    b = int(chunk_blk[k])
                    rows = gathp.tile([128, CATW], f16, tag="rows", bufs=5)
                    nc.gpsimd.indirect_dma_start(
                        out=rows[:], out_offset=None, in_=acatc[:],
                        in_offset=bass.IndirectOffsetOnAxis(
                            ap=srcrow_w[:, k:k + 1], axis=0))
                    # acs2[(c,a)] = ang_a * chi2_src[c]
                    acs = edgep.tile([128, 180], f32, tag="acs", bufs=3)
                    nc.vector.tensor_tensor(
                        acs[:].rearrange("p (c k) -> p c k", c=9),
                        ef["angv"][:, cl].rearrange("p (x k) -> p x k", x=1)
                        .to_broadcast([128, 9, 20]),
                        rows[:, FW:FW + 9].rearrange("p (c x) -> p c x", x=1)
                        .to_broadcast([128, 9, 20]),
                        op=OP.mult)
                    # y2 = acs2 (x) radx_mp  (A_B term; embd applied at drain)
                    y6 = edgep.tile([128, FW], f32, tag="y6", bufs=3)
                    acs5 = acs[:].rearrange("p (c k x) -> p c k x", c=9, x=1)
                    nc.vector.tensor_tensor(
                        y6[:].rearrange("p (c k s) -> p c k s", c=9, s=6),
                        acs5[:].to_broadcast([128, 9, 20, 6]),
                        bass.AP(ef["radx"].tensor,
                                ef["radx"].offset + 240 * cl,
                                [ef["radx"].ap[0], [0, 9], [6, 20], [1, 6]]),
                        op=OP.mult)
                    # mAr = A[src] * radx_ar  (A_r term)
                    mAr = edgep.tile([128, FW], f32, tag="mAr", bufs=3)
                    mv = mAr[:].rearrange("p (c q) -> p c q", c=9)
                    rv = rows[:, :FW].rearrange("p (c q) -> p c q", c=9)
                    arv = bass.AP(ef["radx"].tensor,
                                  ef["radx"].offset + 240 * cl + 120,
                                  [ef["radx"].ap[0], [0, 5], [1, 120]])
                    nc.gpsimd.tensor_tensor(mv[:, :5, :], rv[:, :5, :], arv,
                                            op=OP.mult)
                    arv4 = bass.AP(ef["radx"].tensor,
                                   ef["radx"].offset + 240 * cl + 120,
                                   [ef["radx"].ap[0], [0, 4], [1, 120]])
                    nc.vector.tensor_tensor(mv[:, 5:, :], rv[:, 5:, :], arv4,
                                            op=OP.mult)
                    if k == first_chunk_of_block[b]:
                        psum2[b] = (
                            [psA.tile([128, 360], f32, tag=f"sc{i_}",
                                      name=f"psB{i_}") for i_ in range(3)],
                            [psA.tile([128, 360], f32, tag=f"sc{i_}",
                                      name=f"psR{i_}") for i_ in range(3)])
                    st = (k == first_chunk_of_block[b])
                    sp = (k == last_chunk_of_block[b])
                    for g3 in range(3):
                        nc.tensor.matmul(
                            psum2[b][0][g3][:],
                            ef["Pm"][:, 128 * cl:128 * (cl + 1)],
                            y6[:, 360 * g3:360 * (g3 + 1)],
                            start=st, stop=sp)
                    for g3 in range(3):
                        nc.tensor.matmul(
                            psum2[b][1][g3][:],
                            ef["Pm"][:, 128 * cl:128 * (cl + 1)],
                            mAr[:, 360 * g3:360 * (g3 + 1)],
                            start=st, stop=sp)
                    if not sp:
                        continue
                    # ---- per-block drain ----
                    b_ = b
                    x4 = b_ % 4
                    anew = arow4[:, CATW * x4: CATW * x4 + FW]
                    for g3 in range(3):
                        nc.scalar.activation(anew[:, 360 * g3:360 * (g3 + 1)],
                                             psum2[b_][1][g3][:],
                                             AF.Copy, scale=1.0 / 256.0)
                    # anew += emb_j(d) * psum_y2 ; anew += emb_j(d) * memhat
                    for g3 in range(3):          # bank g3 holds i = g3
                        for j in range(3):
                            cix = 3 * g3 + j
                            ej = embn2_w[:, 3 * b_ + j:3 * b_ + j + 1]
                            ejm = embn_w[:, 3 * b_ + j:3 * b_ + j + 1]
                            sl = anew[:, cix * 120:(cix + 1) * 120]
                            nc.vector.scalar_tensor_tensor(
                                sl, psum2[b_][0][g3][:, 120 * j:120 * (j + 1)],
                                ej, sl, op0=OP.mult, op1=OP.add)
                            mtmp = blkp.tile([128, 120], f32, tag="mtmp",
                                             bufs=3)
                            nc.gpsimd.tensor_tensor(
                                mtmp[:],
                                memhat_all[:, HW * b_ + 120 * g3:
                                           HW * b_ + 120 * (g3 + 1)],
                                ejm.to_broadcast([128, 120]),
                                op=OP.mult)
                            nc.gpsimd.tensor_tensor(sl, sl, mtmp[:], op=OP.add)
                    # ---- quad node phase + MLP + energy ----
                    if b_ == QUADS[quad_of_block[b_]][-1]:
                        q = QUADS[quad_of_block[b_]]
                        nb = len(q)
                        brows = quadp.tile([128, 4 * 324], f32, tag="brows")
                        symmetrize_quad(arow4[:, :nb * CATW], nb,
                                        brows[:, :nb * 324], stride=CATW)
                        for xi, bb in enumerate(q):
                            h1f = bt_compute(brows[:, 324 * xi:324 * (xi + 1)],
                                             bb, stage=1)
                            h1s = blkp.tile([64, 128], f32, tag="h1s")
                            nc.scalar.activation(h1s[:], h1f[:], AF.Silu,
                                                 bias=b1c_w[:])
                            h2p = psT.tile([32, 128], f32, tag="ps1", name="h2p")
                            nc.tensor.matmul(h2p[:], w2_w[:], h1s[:], start=True,
                                             stop=True)
                            h2s = blkp.tile([32, 128], f32, tag="h2s")
                            nc.scalar.activation(h2s[:], h2p[:], AF.Silu,
                                                 bias=b2c_w[:])
                            atp = psT.tile([1, 128], f32, tag="ps1", name="atp")
                            nc.tensor.matmul(atp[:], w3_w[:], h2s[:], start=True,
                                             stop=True)
                            ats = blkp.tile([1, 128], f32, tag="ats")
                            nc.scalar.activation(ats[:], atp[:], AF.Copy,
                                                 bias=b3val)
                            att = psT.tile([128, 16], f32, tag="ps1", name="att")
                            nc.tensor.transpose(att[:, :1], ats[:], ident[:1, :1])
                            atsb = blkp.tile([128, 1], f32, tag="atsb")
                            nc.vector.tensor_copy(atsb[:], att[:, :1])
                            oh = blkp.tile([128, 16], f32, tag="oh")
                            nc.vector.tensor_scalar(oh[:], iota16[:],
                                                    bid_w[:, bb:bb + 1], None,
                                                    op0=OP.is_equal)
                            ep = psT.tile([16, 16], f32, tag="ps1", name="ep")
                            nc.tensor.matmul(ep[:, :1], oh[:], atsb[:],
                                             start=True, stop=True)
                            esb = blkp.tile([16, 1], f32, tag="esb")
                            nc.vector.tensor_copy(esb[:], ep[:, :1])
                            nc.vector.tensor_tensor(energy_sb[:], energy_sb[:],
                                                    esb[:], op=OP.add)
                            if DEBUG:
                                nc.sync.dma_start(
                                    dbg_anew_d[:, FW * bb:FW * (bb + 1)],
                                    arow4[:, CATW * xi:CATW * xi + FW])

            nc.sync.dma_start(energy_d[:], energy_sb[:])

    return nc



def kernel(pos, node_type, src, dst, shifts, batch_ids, Wemb, freqs,
           W_rt, W_mem, W_Ar, W_chi, W1, b1, W2, b2, W3, b3):
    _install_ntff_shim()
    import concourse.mybir as mybir
    from concourse.bass_utils import run_bass_kernel_spmd

    shards, chunk_blk, supers, KCH = host_prepare(
        pos, node_type, src, dst, shifts, batch_ids, Wemb)
    w = host_weights(Wemb, freqs, W_rt, W_mem, W_Ar, W_chi, W1, b1, W2, b2,
                     W3, b3)
    nc = build_program(chunk_blk, supers, KCH, w["b3"])
    _split_waits(nc, mybir)

    common = {k: w[k] for k in ("wl1", "wl2", "multrow", "freqs6", "wchi",
                                "w1", "w2", "w3", "b1c", "b2c")}
    in_maps = []
    for i in range(NCORES):
        m = dict(common)
        m.update(eb=shards[i]["eb"],
                 srcrow=np.ascontiguousarray(shards[i]["srcrow"]),
                 bid=shards[i]["bid"],
                 embn=shards[i]["embn"])
        in_maps.append(m)

    import os
    trace = bool(int(os.environ.get("TRN_TRACE", "0")))
    res = run_bass_kernel_spmd(nc, in_maps, core_ids=list(range(NCORES)),
                               trace=trace)
    energy = np.zeros(N_GRAPHS, np.float32)
    for i in range(NCORES):
        energy += res.results[i]["energy"][:, 0]
    kernel._last_results = res
    return energy

